# revision 81
# baseline (speedup 1.0000x reference)
"""Trainium2 Bass kernel for nn_CrossAttention_38783554683555.

Two cross-attention branches x 4 batches = 8 independent attention units,
one per NeuronCore (pure data parallel, no collectives).

Per core (N=4096, C=64):
  Wk is folded into Wq host-side:  S^T = featE^T (MF featE + MO oth)
    MF = KAP*SCALE*wkE@wqfE^T, MO = KAP*SCALE*wkE@Wo  (the softmax SCALE,
    the DoubleRow doubling, and the KAP fp8-range rescale are undone by the
    exp input scale SIG = 1/(2*KAP))
  Y = MF@featE + MO@oth on PE -> psum -> fp8 y8   (the only q/k crossing)
  S: fp8 DoubleRow matmul with stride-0 middle dims (lhsT = featE8 chunk,
     rhs = y8 slice); psum = 2*S
  E = exp(SIG*S) in fp8e4m3, split across the only two engines the ISA
  allows to read PSUM:
        ACT: native exp activation (fp8 output)
        DVE: Schraudolph bit-trick exp: rint(A8*SIG*s + B8) as int8 bits
             IS the fp8e4m3 encoding (~3% rms; averages out in softmax)
  AV: fp8 DoubleRow matmul pairing two 128-key chunks per pass; V' tile
      stride padded to VCP=80 (dual-fp8 LDWEIGHTS needs 16B-aligned steps);
      denominator rides along as an extra V' column
  tail: rc = 1/av[C] (DVE), partition_broadcast on Pool, o2 = av*rb + feat
  gamma: V' denom col = fp8(1/gamma), numerator scaled by gamma*fp8(1/gamma)
  so the fp8 rounding of 1/gamma cancels exactly.
"""

import numpy as np

import concourse.bass as bass
import concourse.tile as tile
from concourse import bacc, mybir
from concourse.bass_utils import run_bass_kernel_spmd

N = 4096          # H*W
C = 64            # channels
D = 16            # query/key dim after concat
MB = 1024         # m-block (query) size
NMB = N // MB     # 4
NCH = N // 128    # 32 key chunks
NPAIR = NCH // 2  # 16 chunk pairs for DoubleRow AV
B = 4
SCALE = (C // 8) ** -0.5
VC = 66           # live V' columns (64 ch + denom col + pad)
VCP = 80          # padded V' stride: DoubleRow lhsT step must be 16B-aligned
KAP = 4.0         # Y rescale so fp8 Y avoids subnormals; exp scale undoes it
WPK = 2 * (C + 1) + VCP  # packed weights: MF^T | MO^T(+pad row) | wvE

F32 = mybir.dt.float32
BF16 = mybir.dt.bfloat16
F8 = mybir.dt.float8e4
I8 = mybir.dt.int8
EXP = mybir.ActivationFunctionType.Exp
DR = mybir.MatmulPerfMode.DoubleRow
MULT = mybir.AluOpType.mult
ADD = mybir.AluOpType.add

# Schraudolph constants for fp8e4m3 bit-trick exp (executor rounds to nearest)
A8 = float(8.0 * np.log2(np.e))
B8 = 56.0 - 0.35

_CACHE = {}


def build_nc():
    nc = bacc.Bacc("TRN2", target_bir_lowering=False)
    featE = nc.dram_tensor("featE", [C + 1, N], BF16, kind="ExternalInput")
    featE8 = nc.dram_tensor("featE8", [C + 1, N], F8, kind="ExternalInput")
    other = nc.dram_tensor("other", [C, N], BF16, kind="ExternalInput")
    wpack = nc.dram_tensor("wpack", [C + 1, WPK], BF16, kind="ExternalInput")
    out = nc.dram_tensor("out", [C, N], F32, kind="ExternalOutput")

    # --- cumulative load balancer for the three exp-capable engines -------
    # (the Tile scheduler reorders instructions itself; only totals matter)
    COST = {"A": {1024: 1038.0, 512: 612.0, 264: 405.0},
            "D": {1024: 1316.0, 512: 658.0, 264: 400.0},
            "P": {1024: 853.0, 512: 427.0, 264: 220.0}}
    load = {"A": 0.0, "D": 0.0, "P": 0.0}

    def pick(cols=512, allowed="AD"):
        e = min(allowed, key=lambda k: load[k] + COST[k][cols])
        load[e] += COST[e][cols]
        return e

    def charge(e, ns):
        load[e] += ns

    with tile.TileContext(nc) as tc:
        with (
            tc.tile_pool(name="const", bufs=1) as cpool,
            tc.tile_pool(name="emb", bufs=2) as epool,
            tc.tile_pool(name="tail", bufs=2) as tpool,
            tc.tile_pool(name="psA", bufs=2, space="PSUM") as psA,
            tc.tile_pool(name="psD", bufs=2, space="PSUM") as psD,
            tc.tile_pool(name="avpsum", bufs=1, space="PSUM") as avpool,
        ):
            feh = cpool.tile([C + 1, N], BF16)
            fe8 = cpool.tile([C + 1, N], F8)
            oth = cpool.tile([C, N], BF16)
            wpk = cpool.tile([C + 1, WPK], BF16)
            y8 = cpool.tile([C + 1, N], F8)
            vt8 = cpool.tile([128, NCH * VCP], F8)

            mf_s = wpk[:, 0:C + 1]
            mo_s = wpk[0:C, C + 1:2 * (C + 1)]
            wv_s = wpk[:, 2 * (C + 1):2 * (C + 1) + VCP]

            nc.sync.dma_start(wpk[:], wpack[:])
            nc.sync.dma_start(feh[:, 0:512], featE[:, 0:512])
            nc.gpsimd.dma_start(oth[:, 0:512], other[:, 0:512])
            nc.sync.dma_start(feh[:, 512:MB], featE[:, 512:MB])
            nc.gpsimd.dma_start(oth[:, 512:MB], other[:, 512:MB])
            nc.sync.dma_start(fe8[:], featE8[:])
            nc.sync.dma_start(feh[:, MB:], featE[:, MB:])
            nc.sync.dma_start(oth[:, MB:], other[:, MB:])

            POOLS = {"A": psA, "D": psD}

            # ---- prologue emitters (interleaved into the flash loop) -----
            def cv(eng, dst, src):
                # psum fp32 -> sbuf fp8 convert on the chosen engine
                if eng == "A":
                    nc.scalar.copy(dst, src)
                elif eng == "D":
                    nc.vector.tensor_copy(dst, src)
                else:
                    nc.gpsimd.tensor_copy(dst, src)

            def emit_y(j, eng=None, split=False):
                e = eng or pick(1024)
                w = 512 if (split or e == "D") else MB
                for hh in range(MB // w):
                    if split:
                        e = "A" if hh == 0 else "D"
                        charge(e, COST[e][512])
                    ps = POOLS[e].tile([C + 1, w], F32, tag="s",
                                       name=f"psy{j}{hh}")
                    for h2 in range(w // 512):
                        sl = slice(j * MB + hh * w + h2 * 512,
                                   j * MB + hh * w + (h2 + 1) * 512)
                        psl = ps[:, h2 * 512:(h2 + 1) * 512]
                        nc.tensor.matmul(psl, mf_s, feh[:, sl],
                                         start=True, stop=False)
                        nc.tensor.matmul(psl, mo_s, oth[:, sl],
                                         start=False, stop=True)
                    dsl = slice(j * MB + hh * w, j * MB + (hh + 1) * w)
                    cv(e, y8[:, dsl], ps[:])

            def emit_vt(g, eng=None):
                # batch of 4 key chunks -> vt8 cols [g*4*VC, (g+1)*4*VC)
                e = eng or pick(264)
                ps = POOLS[e].tile([128, 4 * VCP], F32, tag="s",
                                   name=f"psv{g}")
                for t in range(4):
                    c = g * 4 + t
                    nc.tensor.matmul(ps[:, t * VCP:(t + 1) * VCP],
                                     feh[:, c * 128:(c + 1) * 128], wv_s,
                                     start=True, stop=True)
                cv(e, vt8[:, g * 4 * VCP:(g + 1) * 4 * VCP], ps[:])

            # minimum before the flash loop, on three parallel engines
            charge("D", 9500.0)    # tail recips+muls are DVE-only
            emit_y(0, split=True)
            emit_vt(0, "D")
            charge("D", 400)
            deferred = {(0, 2): [lambda: emit_vt(1)],
                        (0, 6): [lambda: emit_vt(2)],
                        (0, 10): [lambda: emit_vt(3)],
                        (0, 14): [lambda: emit_vt(4)],
                        (0, 16): [lambda: emit_y(1, split=True)],
                        (0, 18): [lambda: emit_vt(5)],
                        (0, 22): [lambda: emit_vt(6)],
                        (0, 26): [lambda: emit_vt(7)],
                        (1, 10): [lambda: emit_y(2, split=True)],
                        (2, 10): [lambda: emit_y(3, split=True)]}

            # ---- flash loop ----------------------------------------------
            vt_r = vt8[:].rearrange("p (c v) -> p c v", c=NCH)

            def emit_av(av, e_r, mb, j):
                for h in range(2):
                    nc.tensor.matmul(
                        av[0:VCP, h * 512:(h + 1) * 512],
                        vt_r[:, 2 * j:2 * j + 2, :],
                        e_r[:, 2 * j:2 * j + 2, h * 512:(h + 1) * 512],
                        start=(j == 0), stop=(j == NPAIR - 1),
                        perf_mode=DR)

            def emit_tail(av, mb, fin=False):
                # rc = 1/denom on DVE; numerator copy on ACT (parallel);
                # broadcast, multiply, residual-add on Pool (SBUF only)
                w = 512 if fin else MB
                rc = tpool.tile([1, MB], BF16, tag="rc", name="rc")
                rb = tpool.tile([C, MB], BF16, tag="rb", name="rb")
                o1 = tpool.tile([C, MB], F32, tag="o1", name="o1")
                o2 = tpool.tile([C, MB], F32, tag="o2", name="o2")
                fes = feh[0:C, mb * MB:mb * MB + MB]
                qs = [slice(i * w, (i + 1) * w) for i in range(MB // w)]
                with nc.allow_low_precision(reason="denom fits bf16"):
                    for sl in qs:
                        nc.vector.reciprocal(rc[:, sl], av[C:C + 1, sl])
                n1 = None
                if fin:
                    # numerator copy on ACT (idle at the drain) so the
                    # multiplies run on Pool, off the DVE serial spine
                    n1 = tpool.tile([C, MB], BF16, tag="n1", name="n1")
                    with tc.high_priority():
                        for sl in qs:
                            nc.scalar.copy(n1[:, sl], av[0:C, sl])
                for sl in qs:
                    nc.gpsimd.partition_broadcast(rb[:, sl], rc[:, sl])
                for sl in qs:
                    if fin:
                        nc.gpsimd.tensor_mul(o1[:, sl], n1[:, sl], rb[:, sl])
                    else:
                        nc.vector.tensor_mul(o1[:, sl], av[0:C, sl],
                                             rb[:, sl])
                    nc.gpsimd.tensor_add(o2[:, sl], o1[:, sl], fes[:, sl])
                    nc.sync.dma_start(
                        out[:, mb * MB:(mb + 1) * MB][:, sl], o2[:, sl])

            av_tiles = {}
            pend = []          # (mb, j, e_r): AV not yet emitted
            tail_q = []        # tails deferred into the next mb's stream

            SIG = 1.0 / (2.0 * KAP)  # psum -> logit scale (SCALE in MF/MO)

            def emit_exp_chunk(mb, c, e_mb, split=False):
                if split:
                    # last chunks of the final mb: emit per-half so the
                    # tail's h0 chain starts before the h1 exps finish
                    kt = fe8[:, c * 128:(c + 1) * 128]
                    kt_b = kt.unsqueeze(1).broadcast_to([C + 1, 2, 128])
                    for h in range(2):
                        E = pick(512)
                        s = POOLS[E].tile([128, 512], F32, tag="s")
                        q = y8[:, mb * MB + h * 512: mb * MB + (h + 1) * 512]
                        q_b = q.unsqueeze(1).broadcast_to([C + 1, 2, 512])
                        nc.tensor.matmul(s[:], kt_b, q_b,
                                         start=True, stop=True, perf_mode=DR)
                        ev = e_mb[:, c * MB + h * 512:
                                  c * MB + (h + 1) * 512]
                        if E == "A":
                            nc.scalar.activation(ev, s[:], EXP, scale=SIG)
                        else:
                            nc.vector.tensor_scalar(ev.bitcast(I8), s[:],
                                                    A8 * SIG, B8, MULT, ADD)
                    return
                E = pick(1024)
                kt = fe8[:, c * 128:(c + 1) * 128]
                kt_b = kt.unsqueeze(1).broadcast_to([C + 1, 2, 128])

                def s_mm(dst, h):
                    q = y8[:, mb * MB + h * 512: mb * MB + (h + 1) * 512]
                    q_b = q.unsqueeze(1).broadcast_to([C + 1, 2, 512])
                    nc.tensor.matmul(dst, kt_b, q_b,
                                     start=True, stop=True, perf_mode=DR)

                if E == "A":
                    s = psA.tile([128, MB], F32, tag="s")
                    for h in range(2):
                        s_mm(s[:, h * 512:(h + 1) * 512], h)
                    nc.scalar.activation(e_mb[:, c * MB:(c + 1) * MB],
                                         s[:], EXP, scale=SIG)
                else:
                    for h in range(2):
                        s = psD.tile([128, 512], F32, tag="s")
                        s_mm(s[:], h)
                        ev = e_mb[:, c * MB + h * 512:
                                  c * MB + (h + 1) * 512]
                        nc.vector.tensor_scalar(ev.bitcast(I8), s[:],
                                                A8 * SIG, B8, MULT, ADD)

            def flush_av(force):
                while pend and (force or len(pend) >= 2):
                    mb, j, e_r = pend.pop(0)
                    if j == 0:
                        # lazy-alloc so the single av buffer is not reused
                        # while the previous mb still has pending AV matmuls
                        av_tiles[mb] = avpool.tile([128, MB], F32, tag="av",
                                                   name=f"av{mb}")
                    av = av_tiles[mb]
                    emit_av(av, e_r, mb, j)
                    if j == NPAIR - 1:
                        tail_q.append((av_tiles.pop(mb), mb))

            for mb in range(NMB):
                e_mb = epool.tile([128, NCH * MB], F8, tag="e", name=f"e{mb}")
                e_r = e_mb[:].rearrange("p (c q) -> p c q", c=NCH)
                # fast engines for the final chunks of the last mb: they are
                # on the kernel's critical path
                for c in range(NCH):
                    emit_exp_chunk(mb, c, e_mb)
                    if c % 2 == 1:
                        pend.append((mb, c // 2, e_r))
                    for fn in deferred.pop((mb, c), []):
                        fn()
                    if c == 4 and tail_q:
                        # previous mb's tail: emitted here so it never
                        # head-of-line blocks the exp streams
                        emit_tail(*tail_q.pop(0))
                    flush_av(False)
            flush_av(True)
            # final tail split into halves to pipeline its serial chain
            emit_tail(*tail_q.pop(0), fin=True)

    nc.compile()
    return nc


def _prep_core_inputs(inputs):
    """Build the 8 per-core input maps (host-side weight folding)."""
    import ml_dtypes
    bf = ml_dtypes.bfloat16
    f8 = ml_dtypes.float8_e4m3

    x1 = np.asarray(inputs["input1"], np.float32).reshape(B, C, N)
    x2 = np.asarray(inputs["input2"], np.float32).reshape(B, C, N)
    g = lambda k: np.asarray(inputs[k], np.float32)
    wq = [g("wq1"), g("wq2"), g("wq3"), g("wq4")]
    bq = [g("bq1"), g("bq2"), g("bq3"), g("bq4")]
    Z = np.zeros_like(wq[0])
    gamma = float(np.asarray(inputs["gamma"]).reshape(-1)[0])

    # exact-cancel fold for gamma: denom col carries fp8(1/gamma); numerator
    # weights carry gamma*fp8(1/gamma) so the fp8 rounding cancels.
    inv_g8 = float(np.float32(np.asarray(1.0 / gamma, f8)))
    c_corr = gamma * inv_g8

    # q_in1 = [q1, q3, q4, q2];  x3 = (2/3)x1+(1/3)x2, x4 = (1/3)x1+(2/3)x2
    Wf1 = np.vstack([wq[0], (2 / 3) * wq[2], (1 / 3) * wq[3], Z])
    Wo1 = np.vstack([Z, (1 / 3) * wq[2], (2 / 3) * wq[3], wq[1]])
    b1 = np.concatenate([bq[0], bq[2], bq[3], bq[1]])
    # q_in2 = [q2, q4, q3, q1]; feat = x2, other = x1
    Wf2 = np.vstack([wq[1], (2 / 3) * wq[3], (1 / 3) * wq[2], Z])
    Wo2 = np.vstack([Z, (1 / 3) * wq[3], (2 / 3) * wq[2], wq[0]])
    b2 = np.concatenate([bq[1], bq[3], bq[2], bq[0]])

    ones_row = np.ones((1, N), np.float32)

    def branch_weights(r):
        if r == 0:
            Wf, Wo, bb = Wf1, Wo1, b1
            wk_, bk_, wv_, bv_ = g("wk"), g("bk"), g("wv"), g("bv")
        else:
            Wf, Wo, bb = Wf2, Wo2, b2
            wk_, bk_, wv_, bv_ = g("wk2"), g("bk2"), g("wv2"), g("bv2")
        # S = featE^T (MF featE + MO oth); KAP rescale keeps Y in fp8 range,
        # undone by the exp scale. The stride-0 DoubleRow doubling and the
        # softmax SCALE are folded into the exp scale SIG host/device side.
        wkE = np.vstack([wk_.T, bk_[None, :]])                    # [65, 16]
        wqfE = np.vstack([Wf.T, bb[None, :]])                     # [65, 16]
        MF = KAP * SCALE * (wkE @ wqfE.T)                         # [65, 65]
        MO = KAP * SCALE * (wkE[:, :] @ Wo)                       # [65, 64]
        MO = MO[:, :]
        wvE = np.zeros((C + 1, VCP), np.float32)
        wvE[:C, :C] = c_corr * wv_.T
        wvE[C, :C] = c_corr * bv_
        wvE[C, C] = inv_g8
        # device lhsT layout: matmul computes lhsT.T @ rhs
        mfT = MF.T                                   # [65, 65]
        moT = np.vstack([MO.T, np.zeros((1, C + 1), np.float32)])  # [65, 65]
        return np.hstack([mfT, moT, wvE]).astype(bf)

    wsets = [branch_weights(0), branch_weights(1)]
    in_maps = []
    for core in range(8):
        r, b = divmod(core, B)
        feat = x1[b] if r == 0 else x2[b]
        othr = x2[b] if r == 0 else x1[b]
        fE = np.vstack([feat, ones_row])
        in_maps.append({
            "featE": np.ascontiguousarray(fE.astype(bf)),
            "featE8": np.ascontiguousarray(fE.astype(f8)),
            "other": np.ascontiguousarray(othr.astype(bf)),
            "wpack": np.ascontiguousarray(wsets[r]),
        })
    return in_maps


def run(inputs, trace=False, **kw):
    if "nc" not in _CACHE:
        _CACHE["nc"] = build_nc()
    nc = _CACHE["nc"]
    in_maps = _prep_core_inputs(inputs)
    res = run_bass_kernel_spmd(nc, in_maps, list(range(8)), trace=trace, **kw)
    out1 = np.stack([res.results[b]["out"].astype(np.float32)
                     .reshape(C, 64, 64) for b in range(B)])
    out2 = np.stack([res.results[4 + b]["out"].astype(np.float32)
                     .reshape(C, 64, 64) for b in range(B)])
    return (out1, out2), res


def kernel(**inputs):
    (out1, out2), _ = run(inputs)
    return out1, out2
